# revision 2
# baseline (speedup 1.0000x reference)
"""GQA attention block (B=2, T=2048, D=2048, 16 Q heads, 4 KV heads, RoPE,
causal, out-projection) on 8 Trainium2 NeuronCores.

Sharding: core i = (batch b = i//4, kv-group g = i%4). Each core computes the
4 query heads of its kv-group for its batch plus the partial output projection
with the matching 512 rows of wo; the host sums the 4 partials per batch.

Dataflow (per core, bf16 matmuls, fp32 PSUM accumulation):
  Per t-block tb of 512 tokens:
  1. Projections with weights stationary produce Q^T/K^T/V^T directly in
     head-transposed layout [head_dim, t]; head-dim rows are interleaved
     (2j <- j, 2j+1 <- j+64, via host-permuted weight columns) so the RoPE
     half-swap is an adjacent-pair stream_shuffle on DVE.
  2. RoPE on DVE: r = qt*CC + shuffle(qt)*SN, with CC/SN prebuilt [128,T].
  3. V^T is PE-transposed back to natural V (needed as PV stationary).
  4. Attention per head in transposed layout: S^T(sc) = KT_sc.T @ QT (fp32
     PSUM), causal 128x128 triangle mask added on the diagonal strip, strips
     narrowed to the causal column range; exp on ACT writes a bf16 P^T arena;
     PV matmuls accumulate O^T; softmax denominator = bf16 tree-reduce of the
     arena (DVE) + partition_all_reduce (GPSIMD) + reciprocal (DVE).
  5. Output projection: lhsT = O^T chunks, rhs = wo rows; PSUM accumulated
     over the 4 heads, staged to bf16 SBUF, DMA'd out per 128-token chunk.
  The next block's projection matmuls are interleaved into the (ACT-paced)
  attention stream so the PE never idles.
"""

import math

import numpy as np
import ml_dtypes

import concourse.bass as bass
import concourse.bacc as bacc
import concourse.mybir as mybir
from concourse import bass_isa
from concourse.bass_utils import run_bass_kernel_spmd
from concourse.masks import make_identity
from concourse.tile import TileContext

F32 = mybir.dt.float32
BF16 = mybir.dt.bfloat16

D_MODEL = 2048
T = 2048
B = 2
N_HEADS = 16
N_KV = 4
HEAD_DIM = 128
GH = N_HEADS // N_KV      # 4 q heads per core
KD = D_MODEL // 128       # 16 contraction chunks
TB = T // 512             # 4 t-blocks of 512
TC = T // 128             # 16 t-chunks of 128
NEG = -1.0e30
SWAP_MASK = [i ^ 1 for i in range(32)]
MMCHUNK = 4               # projection matmuls per emission thunk


def build_nc(interleave=True) -> bass.Bass:
    nc = bacc.Bacc("TRN2", target_bir_lowering=False)

    w = nc.declare_dram_parameter("w", [128, KD, 768], BF16, isOutput=False)
    wo = nc.declare_dram_parameter("wo", [128, GH, D_MODEL], BF16, isOutput=False)
    xtb = nc.declare_dram_parameter("xtb", [TB, 128, KD * 512], BF16, isOutput=False)
    cc = nc.declare_dram_parameter("cc", [128, T], F32, isOutput=False)
    sn = nc.declare_dram_parameter("sn", [128, T], F32, isOutput=False)
    tri = nc.declare_dram_parameter("tri", [128, 128], BF16, isOutput=False)
    out = nc.declare_dram_parameter("out", [T, D_MODEL], BF16, isOutput=True)

    with TileContext(nc) as tc:
        with (
            tc.tile_pool(name="persist", bufs=1) as persist,
            tc.tile_pool(name="xtp", bufs=2) as xtp,
            tc.tile_pool(name="ropep", bufs=2) as ropep,
            tc.tile_pool(name="qtp", bufs=2) as qtp,
            tc.tile_pool(name="otp", bufs=2) as otp,
            tc.tile_pool(name="arenap", bufs=2) as arenap,
            tc.tile_pool(name="lredp", bufs=2) as lredp,
            tc.tile_pool(name="vtsbp", bufs=2) as vtsbp,
            tc.tile_pool(name="stagep", bufs=2) as stagep,
            tc.tile_pool(name="pacc", bufs=2, space="PSUM") as pacc,
            tc.tile_pool(name="pst", bufs=2, space="PSUM") as pst,
            tc.tile_pool(name="pot", bufs=2, space="PSUM") as pot,
            tc.tile_pool(name="ppo", bufs=2, space="PSUM") as ppo,
        ):
            # ---- resident tensors -------------------------------------
            W = persist.tile([128, KD, 768], BF16)
            WO = persist.tile([128, GH, D_MODEL], BF16)
            CC = persist.tile([128, T], F32)
            SN = persist.tile([128, T], F32)
            TRI = persist.tile([128, 128], BF16)
            ident = persist.tile([128, 128], F32)
            KTs = [persist.tile([128, 512], BF16, name=f"kt{t}") for t in range(TB)]
            Vs = [persist.tile([128, 512], BF16, name=f"v{t}") for t in range(TB)]

            XTs = {}

            def dma_xt(tb):
                xt = xtp.tile([128, KD, 512], BF16, tag="xt", name=f"xt{tb}")
                nc.sync.dma_start(out=xt, in_=xtb[tb])
                XTs[tb] = xt

            # Warmup: keep the PE busy (HAM ramp) while the first DMAs are in
            # flight, and preload the Exp activation table.
            WA = persist.tile([128, 512], BF16)
            WB = persist.tile([128, 16], BF16)
            WC = persist.tile([128, 16], BF16)
            nc.vector.memset(WC, 0.0)
            nc.vector.memset(WB, 0.0)
            nc.gpsimd.memset(WA, 0.0)
            nc.scalar.activation(WC[0:1, 0:8], WC[0:1, 8:16],
                                 mybir.ActivationFunctionType.Exp)
            warm_ps = pot.tile([128, 512], F32, tag="ot", name="warm_ps")
            for i in range(8):
                nc.tensor.matmul(warm_ps[0:16, :], WB, WA,
                                 start=True, stop=True)

            # Cold-start: stream W and xt0 in interleaved per-chunk DMAs so
            # the first projection matmuls start ~1us in instead of ~16us.
            xt0 = xtp.tile([128, KD, 512], BF16, tag="xt", name="xt0")
            XTs[0] = xt0
            for k in range(KD):
                nc.sync.dma_start(out=W[:, k, :], in_=w[:, k, :])
                nc.sync.dma_start(out=xt0[:, k, :],
                                  in_=xtb[0][:, k * 512:(k + 1) * 512])
            nc.sync.dma_start(out=CC[:, 0:512], in_=cc[:, 0:512])
            nc.sync.dma_start(out=SN[:, 0:512], in_=sn[:, 0:512])
            nc.sync.dma_start(out=TRI, in_=tri[:, :])
            make_identity(nc, ident)
            dma_xt(1)
            nc.sync.dma_start(out=WO, in_=wo.rearrange("p h c -> p (h c)"))
            for t in range(1, TB):
                nc.sync.dma_start(out=CC[:, t * 512:(t + 1) * 512],
                                  in_=cc[:, t * 512:(t + 1) * 512])
                nc.sync.dma_start(out=SN[:, t * 512:(t + 1) * 512],
                                  in_=sn[:, t * 512:(t + 1) * 512])

            QT = {}   # (tb, h) -> roped Q^T tile [128, 512] bf16
            OT = {}   # (tb, h) -> O^T tile [128, 512] bf16

            def rope(dst, src_ps, tb, tag):
                cslice = slice(tb * 512, (tb + 1) * 512)
                sh = ropep.tile([128, 512], F32, tag="sh", name=f"sh{tag}")
                m1 = ropep.tile([128, 512], BF16, tag="m1", name=f"m1{tag}")
                m2 = ropep.tile([128, 512], BF16, tag="m2", name=f"m2{tag}")
                nc.vector.stream_shuffle(sh, src_ps, SWAP_MASK)
                nc.vector.tensor_mul(m1, src_ps, CC[:, cslice])
                nc.vector.tensor_mul(m2, sh, SN[:, cslice])
                nc.vector.tensor_add(dst, m1, m2)

            def qproj_fin(tb, h, ps):
                q = qtp.tile([128, 512], BF16, tag=f"qt{h}", name=f"qt{tb}_{h}")
                rope(q, ps, tb, f"q{tb}_{h}")
                QT[(tb, h)] = q

            def vproj_fin(tb, ps):
                vt = vtsbp.tile([128, 512], F32, tag="vt", name=f"vt{tb}")
                nc.scalar.copy(vt, ps)
                tp = ppo.tile([128, 512], F32, tag="po", name=f"vtp{tb}")
                for i in range(4):
                    nc.tensor.transpose(tp[:, i * 128:(i + 1) * 128],
                                        vt[:, i * 128:(i + 1) * 128], ident)
                nc.scalar.copy(Vs[tb], tp)

            def proj0_emit():
                """Projections for t-block 0: k-outer for kt/vt/q0 (paced by
                the interleaved W/xt chunk DMAs), then q1-q3 k-inner once the
                chunks are resident. Staggers the rope/DVE work so attention
                on head 0 can start as soon as possible."""
                xt = XTs[0]
                kt_ps = pacc.tile([128, 512], F32, tag="acc", name="kt_ps0")
                vt_ps = pacc.tile([128, 512], F32, tag="acc", name="vt_ps0")
                q0_ps = pst.tile([128, 512], F32, tag="st", name="q0_ps0")
                for k in range(KD):
                    se = dict(start=(k == 0), stop=(k == KD - 1))
                    nc.tensor.matmul(kt_ps, W[:, k, 512:640], xt[:, k, :], **se)
                    nc.tensor.matmul(vt_ps, W[:, k, 640:768], xt[:, k, :], **se)
                    nc.tensor.matmul(q0_ps, W[:, k, 0:128], xt[:, k, :], **se)
                rope(KTs[0], kt_ps, 0, "k0")
                qproj_fin(0, 0, q0_ps)
                vproj_fin(0, vt_ps)
                later = [pst.tile([128, 512], F32, tag="st", name="q1_ps0"),
                         pot.tile([128, 512], F32, tag="ot", name="q2_ps0"),
                         ppo.tile([128, 512], F32, tag="po", name="q3_ps0")]
                for h in range(1, GH):
                    ps = later[h - 1]
                    for k in range(KD):
                        nc.tensor.matmul(ps, W[:, k, h * 128:(h + 1) * 128],
                                         xt[:, k, :],
                                         start=(k == 0), stop=(k == KD - 1))
                    qproj_fin(0, h, ps)

            def proj_thunks(tb):
                """Emission thunks for projections of t-block tb."""
                thunks = []
                xt = XTs[tb]

                def accum(ps, wlo, whi, k0):
                    for k in range(k0, min(k0 + MMCHUNK, KD)):
                        nc.tensor.matmul(ps, W[:, k, wlo:whi], xt[:, k, :],
                                         start=(k == 0), stop=(k == KD - 1))

                def kproj():
                    ps = pacc.tile([128, 512], F32, tag="acc", name=f"kt_ps{tb}")
                    for k0 in range(0, KD, MMCHUNK):
                        accum(ps, 512, 640, k0)
                    rope(KTs[tb], ps, tb, f"k{tb}")
                thunks.append(kproj)

                def vproj_mm(k0, ps_box):
                    if k0 == 0:
                        ps_box["ps"] = pacc.tile([128, 512], F32, tag="acc",
                                                 name=f"vt_ps{tb}")
                    accum(ps_box["ps"], 640, 768, k0)

                vbox = {}
                for k0 in range(0, KD, MMCHUNK):
                    thunks.append(lambda k0=k0: vproj_mm(k0, vbox))
                thunks.append(lambda: vproj_fin(tb, vbox["ps"]))

                def qproj_mm(h, k0, box):
                    if k0 == 0:
                        box["ps"] = pacc.tile([128, 512], F32, tag="acc",
                                              name=f"qt_ps{tb}_{h}")
                    accum(box["ps"], h * 128, (h + 1) * 128, k0)

                for h in range(GH):
                    box = {}
                    for k0 in range(0, KD, MMCHUNK):
                        thunks.append(lambda h=h, k0=k0, box=box: qproj_mm(h, k0, box))
                    thunks.append(lambda h=h, box=box: qproj_fin(tb, h, box["ps"]))
                return thunks

            def attn_thunks(tb):
                """Emission thunks for attention of t-block tb (all heads)."""
                thunks = []
                nsc = 4 * (tb + 1)

                half = nsc // 2
                q2 = half // 2
                last_tb = tb == TB - 1
                if last_tb:
                    ORD = list(range(4 * tb, nsc)) + list(range(0, 4 * tb))
                else:
                    ORD = list(range(nsc))

                def make_head(h):
                    state = {}

                    def setup():
                        arena = arenap.tile([128, nsc, 512], BF16, tag="arena",
                                            name=f"ar{tb}_{h}")
                        lv1 = arenap.tile([128, half, 512], BF16, tag="lv1",
                                          name=f"lv{tb}_{h}")
                        ot_ps = pot.tile([128, 512], F32, tag="ot",
                                         name=f"ot_ps{tb}_{h}")
                        state["arena"] = arena
                        state["lv1"] = lv1
                        state["ot"] = ot_ps
                        if half >= 4:
                            state["lv2"] = arenap.tile(
                                [128, q2, 512], BF16, tag="lv2",
                                name=f"lw{tb}_{h}")
                        if last_tb:
                            state["lv3"] = arenap.tile(
                                [128, 3, 512], BF16, tag="lv3",
                                name=f"lx{tb}_{h}")
                        for j in range(1, 4):
                            nc.gpsimd.memset(arena[:, 4 * tb + j, 0:j * 128], 0.0)

                    def strip(sc):
                        arena = state["arena"]
                        diag = sc >= 4 * tb
                        lo = (sc - 4 * tb) * 128 if diag else 0
                        st = pst.tile([128, 512], F32, tag="st",
                                      name=f"st{tb}_{h}_{sc}")
                        nc.tensor.matmul(
                            st[:, lo:512],
                            KTs[sc // 4][:, (sc % 4) * 128:(sc % 4 + 1) * 128],
                            QT[(tb, h)][:, lo:512], start=True, stop=True)
                        nc.scalar.activation(arena[:, sc, lo:512], st[:, lo:512],
                                             mybir.ActivationFunctionType.Exp)
                        if diag:
                            nc.gpsimd.tensor_mul(arena[:, sc, lo:lo + 128],
                                                 arena[:, sc, lo:lo + 128], TRI)

                    def pv(sc):
                        arena, ot_ps = state["arena"], state["ot"]
                        diag = sc >= 4 * tb
                        lo = (sc - 4 * tb) * 128 if diag else 0
                        nc.tensor.matmul(
                            ot_ps[:, lo:512],
                            Vs[sc // 4][:, (sc % 4) * 128:(sc % 4 + 1) * 128],
                            arena[:, sc, lo:512],
                            start=(sc == ORD[0]), stop=(sc == ORD[-1]),
                            skip_group_check=True)

                    def lv1add(s):
                        arena, lv1 = state["arena"], state["lv1"]
                        nc.vector.tensor_add(lv1[:, s, :], arena[:, s, :],
                                             arena[:, s + half, :])

                    def lv2add(s):
                        lv1, lv2 = state["lv1"], state["lv2"]
                        nc.vector.tensor_add(lv2[:, s, :], lv1[:, s, :],
                                             lv1[:, s + q2, :])

                    def late_adds(p):
                        # last block (nsc=16): reduce the first 13 emitted
                        # strips into lv3[0] off the critical path; strips
                        # p13/p14 join via one add, p15 joins in finish().
                        arena, lv1 = state["arena"], state["lv1"]
                        lv2, lv3 = state["lv2"], state["lv3"]

                        def A(d, di, s0, i0, s1, i1):
                            nc.vector.tensor_add(d[:, di, :], s0[:, i0, :],
                                                 s1[:, i1, :])
                        if 6 <= p <= 11:
                            A(lv1, p - 6, arena, ORD[p - 6], arena, ORD[p])
                        if p == 9:
                            A(lv2, 0, lv1, 0, lv1, 3)
                        elif p == 10:
                            A(lv2, 1, lv1, 1, lv1, 4)
                        elif p == 11:
                            A(lv2, 2, lv1, 2, lv1, 5)
                            A(lv3, 1, lv2, 0, lv2, 1)
                        elif p == 12:
                            A(lv1, 0, lv3, 1, arena, ORD[12])
                            A(lv3, 0, lv1, 0, lv2, 2)
                        elif p == 14:
                            A(lv3, 2, arena, ORD[13], arena, ORD[14])
                            A(lv3, 0, lv3, 0, lv3, 2)

                    def finish():
                        ot_ps = state["ot"]
                        lv = state["lv2"] if half >= 4 else state["lv1"]
                        m = q2 if half >= 4 else half
                        lsum = lredp.tile([128, 512], F32, tag="lsum",
                                          name=f"lsum{tb}_{h}")
                        if last_tb:
                            for p_ in range(2):
                                cs = slice(p_ * 256, (p_ + 1) * 256)
                                nc.vector.tensor_add(
                                    lsum[:, cs], state["lv3"][:, 0, cs],
                                    state["arena"][:, ORD[-1], cs])
                            m = 0
                        while m > 2:
                            if m % 2:
                                nc.vector.tensor_add(lv[:, 0, :], lv[:, 0, :],
                                                     lv[:, m - 1, :])
                                m -= 1
                            else:
                                h2 = m // 2
                                nc.vector.tensor_add(lv[:, 0:h2, :],
                                                     lv[:, 0:h2, :],
                                                     lv[:, h2:m, :])
                                m = h2
                        if not last_tb:
                            nc.vector.tensor_add(lsum, lv[:, 0, :], lv[:, 1, :])
                        lred = lredp.tile([128, 512], F32, tag="lred",
                                          name=f"lred{tb}_{h}")
                        o = otp.tile([128, 512], BF16, tag=f"ot{h}",
                                     name=f"otb{tb}_{h}")
                        if last_tb:
                            for p in range(2):
                                cs = slice(p * 256, (p + 1) * 256)
                                nc.gpsimd.partition_all_reduce(
                                    lred[:, cs], lsum[:, cs], 128,
                                    bass_isa.ReduceOp.add)
                                nc.vector.reciprocal(lred[:, cs], lred[:, cs])
                                nc.vector.tensor_mul(o[:, cs], ot_ps[:, cs],
                                                     lred[:, cs])
                        else:
                            nc.gpsimd.partition_all_reduce(
                                lred, lsum, 128, bass_isa.ReduceOp.add)
                            for p in range(2):
                                cs = slice(p * 256, (p + 1) * 256)
                                nc.vector.reciprocal(lred[:, cs], lred[:, cs])
                                nc.vector.tensor_mul(o[:, cs], ot_ps[:, cs],
                                                     lred[:, cs])
                        OT[(tb, h)] = o

                    return setup, strip, pv, lv1add, lv2add, late_adds, finish

                for h in range(GH):
                    (setup, strip, pv, lv1add, lv2add, late_adds,
                     finish) = make_head(h)
                    thunks.append(setup)
                    # software pipeline: S/exp runs one strip ahead of PV;
                    # denominator tree levels run as their inputs complete
                    thunks.append(lambda s=strip: s(ORD[0]))
                    for p in range(1, nsc):
                        thunks.append(lambda s=strip, sc=ORD[p]: s(sc))
                        thunks.append(lambda f=pv, sc=ORD[p - 1]: f(sc))
                        if last_tb:
                            thunks.append(lambda a=late_adds, p=p: a(p))
                        elif p >= half:
                            thunks.append(lambda a=lv1add, s=p - half: a(s))
                            s2 = p - half - q2
                            if half >= 4 and 0 <= s2 < q2:
                                thunks.append(lambda a=lv2add, s=s2: a(s))
                    thunks.append(lambda f=pv, sc=ORD[-1]: f(sc))
                    thunks.append(finish)
                return thunks

            def outproj_thunks(tb, dve_copies=False, pools=None):
                thunks = []
                pools = pools or [ppo]

                def copy_out(ti, n, po, stage):
                    if dve_copies or n % 4 == 0:
                        nc.vector.tensor_copy(stage[:, n * 512:(n + 1) * 512], po)
                    else:
                        nc.scalar.copy(stage[:, n * 512:(n + 1) * 512], po)
                    nc.sync.dma_start(
                        out=out[ti * 128:(ti + 1) * 128, n * 512:(n + 1) * 512],
                        in_=stage[:, n * 512:(n + 1) * 512])

                def make_chunk(tc_i):
                    def chunk():
                        ti = tb * 4 + tc_i
                        stage = stagep.tile([128, D_MODEL], BF16, tag="stage",
                                            name=f"stage{ti}")
                        for n in range(4):
                            pool = pools[n % len(pools)]
                            po = pool.tile([128, 512], F32,
                                           tag="po" if pool is ppo else "acc",
                                           name=f"po{ti}_{n}")
                            for h in range(GH):
                                nc.tensor.matmul(
                                    po, OT[(tb, h)][:, tc_i * 128:(tc_i + 1) * 128],
                                    WO[:, h, n * 512:(n + 1) * 512],
                                    start=(h == 0), stop=(h == GH - 1))
                            copy_out(ti, n, po, stage)
                    return chunk

                for tc_i in range(4):
                    thunks.append(make_chunk(tc_i))
                return thunks

            def outproj_final(tb):
                """Final-block out-projection: the h=3 accumulation step waits
                on the last attention finish, so run h=0..2 for the next
                n-block before closing the previous one (2-deep in ppo)."""
                for tc_i in range(4):
                    ti = tb * 4 + tc_i
                    stage = stagep.tile([128, D_MODEL], BF16, tag="stage",
                                        name=f"stage{ti}")
                    pos = {}

                    def start_po(n):
                        pool = pacc if n < 2 else ppo
                        po = pool.tile([128, 512], F32,
                                       tag="acc" if n < 2 else "po",
                                       name=f"po{ti}_{n}")
                        for h in range(3):
                            nc.tensor.matmul(
                                po, OT[(tb, h)][:, tc_i * 128:(tc_i + 1) * 128],
                                WO[:, h, n * 512:(n + 1) * 512],
                                start=(h == 0), stop=False)
                        pos[n] = po

                    def end_po(n):
                        po = pos[n]
                        nc.tensor.matmul(
                            po, OT[(tb, 3)][:, tc_i * 128:(tc_i + 1) * 128],
                            WO[:, 3, n * 512:(n + 1) * 512],
                            start=False, stop=True)
                        cols = slice(n * 512, (n + 1) * 512)
                        nc.scalar.copy(stage[:, cols], po)
                        nc.sync.dma_start(
                            out=out[ti * 128:(ti + 1) * 128, cols],
                            in_=stage[:, cols])

                    start_po(0)
                    start_po(1)
                    end_po(0)
                    start_po(2)
                    end_po(1)
                    start_po(3)
                    end_po(2)
                    end_po(3)

            # ---- emission ---------------------------------------------
            # Schedule: att(tb) is ACT(exp)-paced, so interleave the PE-heavy
            # proj(tb+1) and outproj(tb-1) streams into it as filler.
            proj0_emit()
            for tb in range(TB):
                if tb + 2 < TB:
                    dma_xt(tb + 2)
                att = attn_thunks(tb)
                fill = list(proj_thunks(tb + 1)) if tb + 1 < TB else []
                if tb > 0:
                    fill += outproj_thunks(tb - 1, dve_copies=(tb == TB - 1))
                if interleave and fill:
                    # pace the filler against the first ~60% of the attention
                    # stream so its PSUM slots/copies retire before the tail
                    na, nf = len(att), len(fill)
                    na_eff = na
                    ia = if_ = 0
                    while ia < na or if_ < nf:
                        if (ia * nf <= if_ * na_eff and ia < na) or if_ >= nf:
                            att[ia]()
                            ia += 1
                        else:
                            fill[if_]()
                            if_ += 1
                else:
                    for t in att:
                        t()
                    for t in fill:
                        t()
            outproj_final(TB - 1)

    nc.compile()
    return nc


def _prep_core_inputs(x_b, wq, wk, wv, wo, cc_t, sn_t, trimask, g):
    scale = 1.0 / math.sqrt(HEAD_DIM)
    perm = np.empty(128, np.int64)
    perm[0::2] = np.arange(64)
    perm[1::2] = np.arange(64, 128)

    wq_g = (wq[:, g * 512:(g + 1) * 512] * scale).reshape(D_MODEL, GH, 128)
    wq_g = wq_g[:, :, perm].reshape(D_MODEL, 512)
    wk_g = wk[:, g * 128:(g + 1) * 128][:, perm]
    wv_g = wv[:, g * 128:(g + 1) * 128]
    wcat = np.concatenate([wq_g, wk_g, wv_g], axis=1)            # [D, 768]
    w_t = np.ascontiguousarray(wcat.reshape(KD, 128, 768).transpose(1, 0, 2))

    wo_g = wo[g * 512:(g + 1) * 512, :]                          # [512, D]
    wo_t = np.ascontiguousarray(wo_g.reshape(GH, 128, D_MODEL).transpose(1, 0, 2))

    xt = np.ascontiguousarray(
        x_b.T.reshape(KD, 128, TB, 512).transpose(2, 1, 0, 3)).reshape(
            TB, 128, KD * 512)

    bf = ml_dtypes.bfloat16
    return {
        "w": w_t.astype(bf),
        "wo": wo_t.astype(bf),
        "xtb": xt.astype(bf),
        "cc": cc_t,
        "sn": sn_t,
        "tri": trimask,
    }


def _host_inputs(x, wq, wk, wv, wo, cos, sin):
    cc_t = np.empty((128, T), np.float32)
    sn_t = np.empty((128, T), np.float32)
    cc_t[0::2] = cos.T
    cc_t[1::2] = cos.T
    sn_t[0::2] = -sin.T
    sn_t[1::2] = sin.T
    trimask = np.where(np.arange(128)[None, :] >= np.arange(128)[:, None],
                       1.0, 0.0).astype(ml_dtypes.bfloat16)
    in_maps = []
    for i in range(8):
        b, g = i // 4, i % 4
        in_maps.append(_prep_core_inputs(
            x[b], wq, wk, wv, wo, cc_t, sn_t, trimask, g))
    return in_maps


def kernel(x, wq, wk, wv, wo, cos, sin):
    x = np.asarray(x, np.float32)
    wq = np.asarray(wq, np.float32)
    wk = np.asarray(wk, np.float32)
    wv = np.asarray(wv, np.float32)
    wo = np.asarray(wo, np.float32)
    cos = np.asarray(cos, np.float32)
    sin = np.asarray(sin, np.float32)

    nc = build_nc()
    in_maps = _host_inputs(x, wq, wk, wv, wo, cos, sin)
    res = run_bass_kernel_spmd(nc, in_maps, list(range(8)))
    outs = [np.asarray(res.results[i]["out"], np.float32) for i in range(8)]
    full = np.empty((B, T, D_MODEL), np.float32)
    for b in range(B):
        full[b] = outs[4 * b] + outs[4 * b + 1] + outs[4 * b + 2] + outs[4 * b + 3]
    return full
